# revision 1
# baseline (speedup 1.0000x reference)
"""Causal self-attention Trainium2 kernel (8-core head-parallel tensor parallel).

Strategy:
  - 16 heads split across 8 cores (2 heads each).
  - Host prep: x^T (shared), per-core W_qkv slice (transposed), per-core
    W_proj column-slice (transposed), per-core qkv bias slice.
  - Device (per core, all in a transposed "feature-major" dataflow):
      qkv^T = W_slice^T.T @ x^T   (+bias, via ACT eviction)   [384, B*T]
      per (batch b, head hh, q-chunk of 512):
        S^T[k,q]   = K^T.T @ Q^T            (PE, f32r)
        expS       = exp(0.125 * (S^T + causal_mask))  (DVE mask add on
                     diagonal tiles only, ACT exp eviction)
        [y^T; Z]   = [V | 1]^T.T @ expS     (PE accumulate over k-tiles;
                     row 64 = sum(exp) for free)
        y^T       /= Z                      (DVE mul with DMA-broadcast 1/Z)
      out^T partial = Wp_slice^T.T @ y^T    (PE)  -> DRAM [C, B*T]
  - Host: sum 8 partial out^T, transpose, +b_proj.
"""

import sys

if "/opt/trn_rl_repo" not in sys.path:
    sys.path.insert(0, "/opt/trn_rl_repo")

import numpy as np

# ---- problem constants (hardcoded for the grading harness) ----
B, T, C, H = 2, 2048, 1024, 16
HD = C // H            # 64
N_CORES = 8
HPC = H // N_CORES     # heads per core = 2

# dataflow dtype knobs
_F32R = True           # use float32r fast-path matmuls


def _cfg_full():
    return dict(B=B, T=T, C=C, HPC=HPC, f32r=_F32R)


def build_nc(cfg):
    """Build the single-core SPMD Bass program."""
    import concourse.bacc as bacc
    import concourse.mybir as mybir
    import concourse.tile as tile
    from concourse.masks import make_identity

    Bc, Tc, Cc, hpc = cfg["B"], cfg["T"], cfg["C"], cfg["HPC"]
    f32r = mybir.dt.float32r if cfg["f32r"] else mybir.dt.float32
    f32 = mybir.dt.float32
    bf16 = mybir.dt.bfloat16
    BT = Bc * Tc
    MQ = hpc * HD                 # rows per m-group (q|k|v) = 128
    assert MQ == 128
    KT_C = Cc // 128              # contraction tiles for qkv/x
    TOKC = 512
    NCH = BT // TOKC              # token chunks over both batches
    QC = Tc // TOKC               # q-chunks per batch
    KTT = Tc // 128               # k-tiles per batch
    MO = Cc // 128                # proj output tiles
    CH_PER_B = Tc // TOKC         # chunks per batch

    nc = bacc.Bacc()
    xT = nc.declare_dram_parameter("xT", [Cc, BT], f32r, isOutput=False)
    wqkvT = nc.declare_dram_parameter("wqkvT", [Cc, 3 * MQ], f32r, isOutput=False)
    bqkv = nc.declare_dram_parameter("bqkv", [3 * MQ, 1], f32, isOutput=False)
    wpT = nc.declare_dram_parameter("wpT", [MQ, Cc], bf16, isOutput=False)
    outT = nc.declare_dram_parameter("outT", [Cc, BT], f32, isOutput=True)

    xT_r = xT.rearrange("(kt p) t -> p kt t", p=128)
    wq_r = wqkvT.rearrange("(kt p) m -> p kt m", p=128)
    bq_r = bqkv.rearrange("(g p) o -> p (g o)", p=128)

    AF = mybir.ActivationFunctionType

    with tile.TileContext(nc) as tc:
        with (
            tc.tile_pool(name="consts", bufs=1) as consts,
            tc.tile_pool(name="xpool", bufs=3) as xpool,
            tc.tile_pool(name="spool", bufs=6) as spool,
            tc.tile_pool(name="tpool", bufs=3) as tpool,
            tc.tile_pool(name="ypool", bufs=3) as ypool,
            tc.tile_pool(name="npool", bufs=3) as npool,
            tc.tile_pool(name="opool", bufs=4) as opool,
            tc.tile_pool(name="ps_mm", bufs=4, space="PSUM") as ps_mm,
            tc.tile_pool(name="ps_y", bufs=2, space="PSUM") as ps_y,
            tc.tile_pool(name="ps_aux", bufs=2, space="PSUM") as ps_aux,
        ):
            # ---- constants ----
            w_sb = consts.tile([128, KT_C, 3 * MQ], f32r, tag="w")
            nc.sync.dma_start(out=w_sb, in_=wq_r)
            b_sb = consts.tile([128, 3], f32, tag="b")
            nc.sync.dma_start(out=b_sb, in_=bq_r)
            wp_sb = consts.tile([128, Cc], bf16, tag="wp")
            nc.sync.dma_start(out=wp_sb, in_=wpT[:, :])
            ident = consts.tile([128, 128], f32, tag="ident")
            make_identity(nc, ident)
            # 4 causal mask tiles (additive, 0 keep / -1e30 drop), offset o =
            # k0-q0 in {0,128,256,384}: keep iff q >= k iff f - p - o >= 0.
            masks = consts.tile([128, 4, TOKC], f32, tag="masks")
            for oi in range(4):
                m = masks[:, oi, :]
                nc.gpsimd.memset(m, 0.0)
                nc.gpsimd.affine_select(
                    out=m, in_=m,
                    compare_op=mybir.AluOpType.is_ge,
                    fill=-1e30,
                    base=-(oi * 128),
                    pattern=[[1, TOKC]],
                    channel_multiplier=-1,
                )

            # qkv^T buffers: q and k feature-major [128, BT]
            ones_f32 = consts.tile([128, HD], f32, tag="ones_f")
            nc.vector.memset(ones_f32[:, :], 1.0)
            ones_sb = consts.tile([1, HD], f32r, tag="ones")
            ident_bf = consts.tile([128, 128], bf16, tag="ident_bf")
            nc.vector.tensor_copy(ident_bf[:, :], ident[:, :])
            nc.scalar.activation(out=ones_sb[:, :], in_=ones_f32[0:1, :],
                                 func=AF.Copy)
            qT_sb = consts.tile([128, BT], f32r, tag="qT")
            kT_sb = consts.tile([128, BT], f32r, tag="kT")
            vT_sb = consts.tile([128, BT], bf16, tag="vT")

            # ---- phase 1: QKV projection (feature-major) ----
            for ch in range(NCH):
                x_t = xpool.tile([128, KT_C, TOKC], f32r, tag="x")
                nc.sync.dma_start(out=x_t, in_=xT_r[:, :, ch * TOKC:(ch + 1) * TOKC])
                b_idx = ch // CH_PER_B
                bcol = (ch % CH_PER_B) * TOKC
                for m in range(3):
                    ps = ps_mm.tile([128, TOKC], f32, tag="mm")
                    for kt in range(KT_C):
                        nc.tensor.matmul(
                            ps[:, :],
                            w_sb[:, kt, m * MQ:(m + 1) * MQ],
                            x_t[:, kt, :],
                            start=(kt == 0), stop=(kt == KT_C - 1),
                        )
                    dst = (qT_sb, kT_sb, vT_sb)[m]
                    nc.scalar.activation(
                        out=dst[:, ch * TOKC:(ch + 1) * TOKC], in_=ps[:, :],
                        func=AF.Identity, bias=b_sb[:, m:m + 1], scale=1.0,
                    )

            # ---- phase 2: V transpose -> per b: [128, kt, 2*65] f32r ----
            # cols [hh*65 : hh*65+64] = V rows of head hh, col hh*65+64 = 1.0
            v_sb = [
                consts.tile([128, KTT, 2 * 65], bf16, tag=f"v{b}",
                            name=f"v{b}") for b in range(Bc)
            ]
            for b in range(Bc):
                for kt in range(KTT):
                    for hh in range(hpc):
                        nc.scalar.activation(
                            out=v_sb[b][:, kt, hh * 65 + 64:hh * 65 + 65],
                            in_=ones_f32[:, 0:1], func=AF.Copy,
                        )
                    ps_t = ps_aux.tile([128, 128], bf16, tag="aux")
                    nc.tensor.transpose(
                        ps_t[:, :],
                        vT_sb[:, b * Tc + kt * 128:b * Tc + (kt + 1) * 128],
                        ident_bf[:, :],
                    )
                    for hh in range(hpc):
                        nc.scalar.activation(
                            out=v_sb[b][:, kt, hh * 65:hh * 65 + 64],
                            in_=ps_t[:, hh * HD:(hh + 1) * HD],
                            func=AF.Copy,
                        )

            # ---- phase 3: attention + phase 4: projection, per (b, q-chunk) ----
            for b in range(Bc):
                for qc in range(QC):
                    yT_t = ypool.tile([128, TOKC], bf16, tag="yT")
                    q_sl = slice(b * Tc + qc * TOKC, b * Tc + (qc + 1) * TOKC)
                    for hh in range(hpc):
                        n_kt = (qc + 1) * (TOKC // 128)
                        psy = ps_y.tile([65, TOKC], f32, tag="y")
                        # software-pipelined S / AV emission
                        exp_tiles = {}

                        def emit_S(kt):
                            pss = ps_mm.tile([128, TOKC], f32, tag="mm")
                            nc.tensor.matmul(
                                pss[:, :],
                                kT_sb[hh * HD:(hh + 1) * HD,
                                      b * Tc + kt * 128:b * Tc + (kt + 1) * 128],
                                qT_sb[hh * HD:(hh + 1) * HD, q_sl],
                                start=True, stop=True,
                            )
                            e_t = spool.tile([128, TOKC], bf16, tag="e")
                            di = kt - qc * (TOKC // 128)
                            if di >= 0:  # diagonal tile: add causal mask first
                                tmp = tpool.tile([128, TOKC], f32, tag="tmp")
                                nc.vector.tensor_add(tmp[:, :], pss[:, :],
                                                     masks[:, di, :])
                                src = tmp
                            else:
                                src = pss
                            nc.scalar.activation(out=e_t[:, :], in_=src[:, :],
                                                 func=AF.Exp, scale=0.125)
                            exp_tiles[kt] = e_t

                        def emit_AV(kt):
                            nc.tensor.matmul(
                                psy[:, :],
                                v_sb[b][:, kt, hh * 65:(hh + 1) * 65],
                                exp_tiles.pop(kt)[:, :],
                                start=(kt == 0), stop=(kt == n_kt - 1),
                            )

                        DEPTH = 3
                        for kt in range(n_kt):
                            emit_S(kt)
                            if kt >= DEPTH:
                                emit_AV(kt - DEPTH)
                        for kt in range(max(0, n_kt - DEPTH), n_kt):
                            emit_AV(kt)

                        # normalize: y^T[:, q] /= Z[q]  (PE rank-1 broadcast)
                        rc = npool.tile([1, TOKC], f32r, tag="rc")
                        with nc.allow_low_precision(reason="1/Z in f32r feeds PE broadcast"):
                            nc.vector.reciprocal(rc[:, :], psy[64:65, :])
                        ps_bc = ps_aux.tile([HD, TOKC], f32, tag="aux")
                        nc.tensor.matmul(ps_bc[:, :], ones_sb[:, :], rc[:, :],
                                         start=True, stop=True)
                        rc_bc = npool.tile([HD, TOKC], f32, tag="rcb")
                        nc.scalar.activation(out=rc_bc[:, :], in_=ps_bc[:, :],
                                             func=AF.Copy)
                        nc.vector.tensor_mul(
                            yT_t[hh * HD:(hh + 1) * HD, :],
                            psy[0:HD, :], rc_bc[:, :],
                        )

                    # projection for this (b, q-chunk)
                    for mo in range(MO):
                        pso = ps_mm.tile([128, TOKC], f32, tag="mm")
                        nc.tensor.matmul(
                            pso[:, :],
                            wp_sb[:, mo * 128:(mo + 1) * 128],
                            yT_t[:, :],
                            start=True, stop=True,
                        )
                        o_t = opool.tile([128, TOKC], f32, tag="o")
                        nc.vector.tensor_copy(o_t[:, :], pso[:, :])
                        nc.sync.dma_start(
                            out=outT[mo * 128:(mo + 1) * 128, q_sl],
                            in_=o_t[:, :],
                        )

    nc.finalize()
    return nc


def prep_inputs(cfg, x, W_attn, b_attn, W_proj, b_proj):
    """Host-side sharding: returns per-core input dicts."""
    Bc, Tc, Cc, hpc = cfg["B"], cfg["T"], cfg["C"], cfg["HPC"]
    n_cores = (Cc // HD) // hpc
    BT = Bc * Tc
    MQ = hpc * HD

    x = np.ascontiguousarray(x, dtype=np.float32)
    xT = np.ascontiguousarray(x.reshape(BT, Cc).T)

    in_maps = []
    for c in range(n_cores):
        r0 = c * MQ
        rows = []
        for g in range(3):
            rows.append(np.arange(g * Cc + r0, g * Cc + r0 + MQ))
        rows = np.concatenate(rows)
        w_slice = W_attn[rows, :]                       # [384, C]
        wqkvT = np.ascontiguousarray(w_slice.T)         # [C, 384]
        bq = np.ascontiguousarray(b_attn[rows].reshape(MQ * 3, 1))
        import ml_dtypes
        wpT = np.ascontiguousarray(W_proj[:, r0:r0 + MQ].T).astype(ml_dtypes.bfloat16)
        in_maps.append({
            "xT": xT,
            "wqkvT": wqkvT.astype(np.float32),
            "bqkv": bq.astype(np.float32),
            "wpT": wpT,
        })
    return in_maps


def combine(cfg, results, b_proj):
    Bc, Tc, Cc = cfg["B"], cfg["T"], cfg["C"]
    acc = results[0]["outT"].astype(np.float32).copy()
    for r in results[1:]:
        acc += r["outT"]
    out = acc.T + b_proj[None, :]
    return np.ascontiguousarray(out.reshape(Bc, Tc, Cc).astype(np.float32))


_NC_CACHE = {}


def kernel(x, W_attn, b_attn, W_proj, b_proj):
    from concourse.bass_utils import run_bass_kernel_spmd

    cfg = _cfg_full()
    key = "full"
    if key not in _NC_CACHE:
        _NC_CACHE[key] = build_nc(cfg)
    nc = _NC_CACHE[key]
    in_maps = prep_inputs(cfg, np.asarray(x), np.asarray(W_attn),
                          np.asarray(b_attn), np.asarray(W_proj),
                          np.asarray(b_proj))
    res = run_bass_kernel_spmd(nc, in_maps, list(range(N_CORES)))
    return combine(cfg, res.results, np.asarray(b_proj, dtype=np.float32))



# revision 2
# speedup vs baseline: 1.1815x; 1.1815x over previous
"""Causal self-attention Trainium2 kernel (8-core head-parallel tensor parallel).

Strategy (v2):
  - 16 heads split across 8 cores (2 heads each), all-bf16 PE dataflow.
  - Host prep: x^T bf16 (shared), per-core W_qkv slice bf16, per-core
    W_proj column-slice bf16, per-core qkv bias slice f32.
  - Device (per core, feature-major dataflow):
      qkv^T = W^T.T @ x^T (bf16 matmul, +bias via DVE tensor_scalar_add)
      per (batch b, head hh, q-chunk of 512):
        S^T[k,q] = K^T.T @ Q^T  (bf16; diagonal tiles column-trimmed)
        expS     = exp(0.125 * S^T) (ACT), causal zeroing via gpsimd
                   affine_select on the diagonal 128-col triangle
        [y^T; Z] = [V | 1]^T.T @ expS  (PE accumulate; row 64 = Z)
        1/Z via ACT ln + PE broadcast + ACT exp(-x)  (no DVE reciprocal)
        y^T      = psy * (1/Z)  (DVE)
      out^T partial = Wp^T.T @ y^T -> bf16 -> DRAM [C, B*T]
  - Normalize + projection emission is deferred into the next head's
    S/AV stream so the PE never idles (keeps the HAM clock warm).
  - Host: sum 8 bf16 partial out^T in f32, transpose, +b_proj.
"""

import sys

if "/opt/trn_rl_repo" not in sys.path:
    sys.path.insert(0, "/opt/trn_rl_repo")

import numpy as np

# ---- problem constants (hardcoded for the grading harness) ----
B, T, C, H = 2, 2048, 1024, 16
HD = C // H            # 64
N_CORES = 8
HPC = H // N_CORES     # heads per core = 2


def _cfg_full():
    return dict(B=B, T=T, C=C, HPC=HPC)


def build_nc(cfg):
    """Build the single-core SPMD Bass program."""
    import concourse.bacc as bacc
    import concourse.mybir as mybir
    import concourse.tile as tile
    from concourse.masks import make_identity

    Bc, Tc, Cc, hpc = cfg["B"], cfg["T"], cfg["C"], cfg["HPC"]
    f32r = mybir.dt.float32r
    f32 = mybir.dt.float32
    bf16 = mybir.dt.bfloat16
    BT = Bc * Tc
    MQ = hpc * HD                 # rows per m-group (q|k|v) = 128
    assert MQ == 128
    KT_C = Cc // 128              # contraction tiles for qkv/x
    TOKC = 512
    NCH = BT // TOKC              # token chunks over both batches
    QC = Tc // TOKC               # q-chunks per batch
    KTT = Tc // 128               # k-tiles per batch
    MO = Cc // 128                # proj output tiles
    CH_PER_B = Tc // TOKC         # chunks per batch
    DTILE = TOKC // 128           # 4 diagonal tiles per q-chunk

    nc = bacc.Bacc()
    xT = nc.declare_dram_parameter("xT", [Cc, BT], bf16, isOutput=False)
    wqkvT = nc.declare_dram_parameter("wqkvT", [Cc, 3 * MQ], bf16, isOutput=False)
    bqkv = nc.declare_dram_parameter("bqkv", [3 * MQ, 1], f32, isOutput=False)
    wpT = nc.declare_dram_parameter("wpT", [MQ, Cc], bf16, isOutput=False)
    outT = nc.declare_dram_parameter("outT", [Cc, BT], bf16, isOutput=True)

    xT_r = xT.rearrange("(kt p) t -> p kt t", p=128)
    wq_r = wqkvT.rearrange("(kt p) m -> p kt m", p=128)
    bq_r = bqkv.rearrange("(g p) o -> p (g o)", p=128)
    outT_r = outT.rearrange("(mo p) t -> p mo t", p=128)

    AF = mybir.ActivationFunctionType

    with tile.TileContext(nc) as tc:
        with (
            tc.tile_pool(name="consts", bufs=1) as consts,
            tc.tile_pool(name="xpool", bufs=3) as xpool,
            tc.tile_pool(name="spool", bufs=6) as spool,
            tc.tile_pool(name="ypool", bufs=3) as ypool,
            tc.tile_pool(name="npool", bufs=3) as npool,
            tc.tile_pool(name="opool", bufs=2) as opool,
            tc.tile_pool(name="ps_mm", bufs=4, space="PSUM") as ps_mm,
            tc.tile_pool(name="ps_y", bufs=2, space="PSUM") as ps_y,
            tc.tile_pool(name="ps_aux", bufs=2, space="PSUM") as ps_aux,
        ):
            # ---- constants ----
            w_sb = consts.tile([128, KT_C, 3 * MQ], bf16, tag="w")
            for kt in range(KT_C):
                nc.sync.dma_start(out=w_sb[:, kt, :], in_=wq_r[:, kt, :])
            b_sb = consts.tile([128, 3], f32, tag="b")
            nc.sync.dma_start(out=b_sb, in_=bq_r)
            wp_sb = consts.tile([128, Cc], bf16, tag="wp")
            nc.sync.dma_start(out=wp_sb, in_=wpT[:, :])
            ident = consts.tile([128, 128], f32, tag="ident")
            make_identity(nc, ident)
            ident_bf = consts.tile([128, 128], bf16, tag="ident_bf")
            nc.vector.tensor_copy(ident_bf[:, :], ident[:, :])

            ones_f32 = consts.tile([128, HD], f32, tag="ones_f")
            nc.vector.memset(ones_f32[:, :], 1.0)
            ones_sb = consts.tile([1, HD], f32r, tag="ones")
            nc.scalar.activation(out=ones_sb[:, :], in_=ones_f32[0:1, :],
                                 func=AF.Copy)
            qT_sb = consts.tile([128, BT], bf16, tag="qT")
            kT_sb = consts.tile([128, BT], bf16, tag="kT")
            vT_sb = consts.tile([128, BT], bf16, tag="vT")

            # V in token-major layout: per b [128(tok), kt, 2*65] where
            # cols hh*65..hh*65+63 = V of head hh, col hh*65+64 = 1.0 (Z row)
            v_sb = [
                consts.tile([128, KTT, 2 * 65], bf16, tag=f"v{b}",
                            name=f"v{b}") for b in range(Bc)
            ]
            for b in range(Bc):
                for hh in range(hpc):
                    nc.vector.memset(v_sb[b][:, :, hh * 65 + 64:hh * 65 + 65], 1.0)

            # ---- phase 1: QKV projection (feature-major, bf16) ----
            for ch in range(NCH):
                x_t = xpool.tile([128, KT_C, TOKC], bf16, tag="x")
                if ch == 0:
                    # split the first chunk's DMA per-kt so compute starts
                    # after ~128KB instead of ~1MB
                    for kt in range(KT_C):
                        nc.sync.dma_start(
                            out=x_t[:, kt, :],
                            in_=xT_r[:, kt, 0:TOKC],
                        )
                else:
                    nc.sync.dma_start(
                        out=x_t, in_=xT_r[:, :, ch * TOKC:(ch + 1) * TOKC])
                for m in range(3):
                    ps = ps_mm.tile([128, TOKC], f32, tag="mm")
                    for kt in range(KT_C):
                        nc.tensor.matmul(
                            ps[:, :],
                            w_sb[:, kt, m * MQ:(m + 1) * MQ],
                            x_t[:, kt, :],
                            start=(kt == 0), stop=(kt == KT_C - 1),
                        )
                    dst = (qT_sb, kT_sb, vT_sb)[m]
                    with nc.allow_low_precision(reason="qkv evict to bf16"):
                        nc.vector.tensor_scalar_add(
                            dst[:, ch * TOKC:(ch + 1) * TOKC], ps[:, :],
                            b_sb[:, m:m + 1],
                        )

            # ---- phase 2: V transpose -> token-major v_sb ----
            for b in range(Bc):
                for kt in range(KTT):
                    ps_t = ps_aux.tile([128, 128], bf16, tag="aux")
                    nc.tensor.transpose(
                        ps_t[:, :],
                        vT_sb[:, b * Tc + kt * 128:b * Tc + (kt + 1) * 128],
                        ident_bf[:, :],
                    )
                    for hh in range(hpc):
                        nc.vector.tensor_copy(
                            v_sb[b][:, kt, hh * 65:hh * 65 + 64],
                            ps_t[:, hh * HD:(hh + 1) * HD],
                        )

            # ---- phase 3+4: attention + projection, software-pipelined ----
            blocks = [(b, qc) for b in range(Bc) for qc in range(QC)]

            def make_norm_tail(psy, yT_t, hh):
                """Returns emitter for: bcast(lnZ) -> exp(-x) -> mul.
                Called later, injected into a subsequent PE stream."""
                lnZ_t = npool.tile([1, TOKC], f32r, tag="lnz")
                nc.scalar.activation(out=lnZ_t[:, :], in_=psy[64:65, :],
                                     func=AF.Ln)

                def emit():
                    ps_bc = ps_aux.tile([HD, TOKC], f32, tag="aux")
                    nc.tensor.matmul(ps_bc[:, :], ones_sb[:, :], lnZ_t[:, :],
                                     start=True, stop=True)
                    inv_bc = npool.tile([HD, TOKC], f32, tag="invz")
                    nc.scalar.activation(out=inv_bc[:, :], in_=ps_bc[:, :],
                                         func=AF.Exp, scale=-1.0)
                    with nc.allow_low_precision(reason="yT in bf16"):
                        nc.vector.tensor_mul(
                            yT_t[hh * HD:(hh + 1) * HD, :],
                            psy[0:HD, :], inv_bc[:, :],
                        )
                return emit

            def make_proj(yT_t, b, qc):
                q_sl = slice(b * Tc + qc * TOKC, b * Tc + (qc + 1) * TOKC)

                def emit():
                    o_t = opool.tile([128, MO, TOKC], bf16, tag="o")
                    for mo in range(MO):
                        pso = ps_mm.tile([128, TOKC], f32, tag="mm")
                        nc.tensor.matmul(
                            pso[:, :],
                            wp_sb[:, mo * 128:(mo + 1) * 128],
                            yT_t[:, :],
                            start=True, stop=True,
                        )
                        with nc.allow_low_precision(reason="partials in bf16"):
                            nc.vector.tensor_copy(o_t[:, mo, :], pso[:, :])
                    nc.sync.dma_start(out=outT_r[:, :, q_sl], in_=o_t)
                return emit

            # deferred work queue: emitters injected into later PE streams
            pending = []   # list of callables

            def emit_head(b, qc, hh, yT_t):
                n_kt = (qc + 1) * DTILE
                q0 = b * Tc + qc * TOKC
                psy = ps_y.tile([65, TOKC], f32, tag="y")
                exp_tiles = {}

                def emit_S(kt):
                    di = kt - qc * DTILE
                    c0 = max(0, di) * 128       # local column start
                    W = TOKC - c0
                    pss = ps_mm.tile([128, TOKC], f32, tag="mm")
                    nc.tensor.matmul(
                        pss[:, 0:W],
                        kT_sb[hh * HD:(hh + 1) * HD,
                              b * Tc + kt * 128:b * Tc + (kt + 1) * 128],
                        qT_sb[hh * HD:(hh + 1) * HD, q0 + c0:q0 + TOKC],
                        start=True, stop=True,
                    )
                    e_t = spool.tile([128, TOKC], bf16, tag="e")
                    nc.scalar.activation(out=e_t[:, 0:W], in_=pss[:, 0:W],
                                         func=AF.Exp, scale=0.125)
                    if di >= 0:
                        # causal zeroing of the leading 128-col triangle:
                        # keep iff f_local >= p
                        nc.gpsimd.affine_select(
                            out=e_t[:, 0:128], in_=e_t[:, 0:128],
                            compare_op=mybir.AluOpType.is_ge,
                            fill=0.0,
                            base=0,
                            pattern=[[1, 128]],
                            channel_multiplier=-1,
                        )
                    exp_tiles[kt] = (e_t, c0, W)

                def emit_AV(kt):
                    e_t, c0, W = exp_tiles.pop(kt)
                    nc.tensor.matmul(
                        psy[:, c0:TOKC],
                        v_sb[b][:, kt, hh * 65:(hh + 1) * 65],
                        e_t[:, 0:W],
                        start=(kt == 0), stop=(kt == n_kt - 1),
                    )

                DEPTH = 3
                for kt in range(n_kt):
                    emit_S(kt)
                    # drain deferred norm/proj work into this PE stream
                    if kt == 2 and pending:
                        pending.pop(0)()
                    elif kt == 4 and pending:
                        pending.pop(0)()
                    if kt >= DEPTH:
                        emit_AV(kt - DEPTH)
                for kt in range(max(0, n_kt - DEPTH), n_kt):
                    emit_AV(kt)
                # lnZ on ACT right away (cheap, no PE dependency)
                pending.append(make_norm_tail(psy, yT_t, hh))

            prev = None   # (yT_t, b, qc) of previous block
            for b, qc in blocks:
                yT_t = ypool.tile([128, TOKC], bf16, tag="yT")
                emit_head(b, qc, 0, yT_t)
                if prev is not None:
                    pending.append(make_proj(*prev))
                emit_head(b, qc, 1, yT_t)
                prev = (yT_t, b, qc)
            # flush tail
            pending.append(make_proj(*prev))
            while pending:
                pending.pop(0)()

    nc.finalize()
    return nc


def prep_inputs(cfg, x, W_attn, b_attn, W_proj, b_proj):
    """Host-side sharding: returns per-core input dicts."""
    import ml_dtypes
    Bc, Tc, Cc, hpc = cfg["B"], cfg["T"], cfg["C"], cfg["HPC"]
    n_cores = (Cc // HD) // hpc
    BT = Bc * Tc
    MQ = hpc * HD

    x = np.ascontiguousarray(x, dtype=np.float32)
    xT = np.ascontiguousarray(x.reshape(BT, Cc).T).astype(ml_dtypes.bfloat16)

    in_maps = []
    for c in range(n_cores):
        r0 = c * MQ
        rows = []
        for g in range(3):
            rows.append(np.arange(g * Cc + r0, g * Cc + r0 + MQ))
        rows = np.concatenate(rows)
        w_slice = W_attn[rows, :]                       # [384, C]
        wqkvT = np.ascontiguousarray(w_slice.T).astype(ml_dtypes.bfloat16)
        bq = np.ascontiguousarray(b_attn[rows].reshape(MQ * 3, 1))
        wpT = np.ascontiguousarray(W_proj[:, r0:r0 + MQ].T).astype(ml_dtypes.bfloat16)
        in_maps.append({
            "xT": xT,
            "wqkvT": wqkvT,
            "bqkv": bq.astype(np.float32),
            "wpT": wpT,
        })
    return in_maps


def combine(cfg, results, b_proj):
    Bc, Tc, Cc = cfg["B"], cfg["T"], cfg["C"]
    acc = results[0]["outT"].astype(np.float32)
    for r in results[1:]:
        acc += r["outT"].astype(np.float32)
    out = acc.T + b_proj[None, :]
    return np.ascontiguousarray(out.reshape(Bc, Tc, Cc).astype(np.float32))


_NC_CACHE = {}


def kernel(x, W_attn, b_attn, W_proj, b_proj):
    from concourse.bass_utils import run_bass_kernel_spmd

    cfg = _cfg_full()
    key = "full"
    if key not in _NC_CACHE:
        _NC_CACHE[key] = build_nc(cfg)
    nc = _NC_CACHE[key]
    in_maps = prep_inputs(cfg, np.asarray(x), np.asarray(W_attn),
                          np.asarray(b_attn), np.asarray(W_proj),
                          np.asarray(b_proj))
    res = run_bass_kernel_spmd(nc, in_maps, list(range(N_CORES)))
    return combine(cfg, res.results, np.asarray(b_proj, dtype=np.float32))


# revision 4
# speedup vs baseline: 1.5706x; 1.3294x over previous
"""Causal self-attention Trainium2 kernel (8-core head-parallel tensor parallel).

Strategy (v3):
  - 16 heads split across 8 cores (2 heads each), all-bf16 PE dataflow.
  - qkv^T = W^T.T @ x^T (bf16), bias added during ACT eviction.
  - Attention per (b, head, q-chunk of 512), feature-major:
      S^T[k,q] = K^T.T @ Q^T  (bf16; diagonal tiles column-trimmed).
      Causal mask: extra PE matmul accumulating Ltri(-1e30) @ I into the
      S PSUM group (no DVE/gpsimd in the dependency chain).
      expS = exp(0.125 * S^T) on ACT -> bf16.
      [y^T; Z] = [V | 1]^T.T @ expS  (PE accumulate; row 64 = Z).
      1/Z = exp(-ln(Z)) on ACT; all activation funcs pinned to the
      natural_log_exp_and_others table set (one ACT_TABLE_LOAD total).
      y^T = psy * bcast(1/Z) (DVE).
  - Normalize + projection emission deferred into later PE streams so the
    PE never idles (keeps the HAM clock warm).
  - out^T partial = Wp^T.T @ y^T -> bf16 partials -> DRAM; host sums.
"""

import sys

if "/opt/trn_rl_repo" not in sys.path:
    sys.path.insert(0, "/opt/trn_rl_repo")

import numpy as np

# ---- problem constants (hardcoded for the grading harness) ----
B, T, C, H = 2, 2048, 1024, 16
HD = C // H            # 64
N_CORES = 8
HPC = H // N_CORES     # heads per core = 2


def _cfg_full():
    return dict(B=B, T=T, C=C, HPC=HPC)


def _make_bacc():
    """Bacc with all activation funcs pinned to the one table set that
    contains both exp and ln, so no ACT_TABLE_LOAD thrash."""
    import concourse.bacc as bacc
    import bass_rust as _bass_rust
    from concourse.hw_specs import get_activation_tables

    class BaccPinnedAct(bacc.Bacc):
        def insert_act_table_loads(self):
            tables = list(get_activation_tables(self.m.arch).items())
            doctored = []
            for name, fns in tables:
                if name == "natural_log_exp_and_others":
                    doctored.append((name, fns))
                else:
                    doctored.append((name, set()))
            _bass_rust.insert_act_table_loads(self, doctored)

    return BaccPinnedAct()


def build_nc(cfg):
    """Build the single-core SPMD Bass program."""
    import concourse.mybir as mybir
    import concourse.tile as tile
    from concourse.masks import make_identity

    Bc, Tc, Cc, hpc = cfg["B"], cfg["T"], cfg["C"], cfg["HPC"]
    f32r = mybir.dt.float32r
    f32 = mybir.dt.float32
    bf16 = mybir.dt.bfloat16
    BT = Bc * Tc
    MQ = hpc * HD                 # rows per m-group (q|k|v) = 128
    assert MQ == 128
    KT_C = Cc // 128              # contraction tiles for qkv/x
    TOKC = 512
    NCH = BT // TOKC              # token chunks over both batches
    QC = Tc // TOKC               # q-chunks per batch
    KTT = Tc // 128               # k-tiles per batch
    MO = Cc // 128                # proj output tiles
    DTILE = TOKC // 128           # 4 diagonal tiles per q-chunk

    nc = _make_bacc()
    xT = nc.declare_dram_parameter("xT", [Cc, BT], bf16, isOutput=False)
    wqkvT = nc.declare_dram_parameter("wqkvT", [Cc, 3 * MQ], bf16, isOutput=False)
    bqkv = nc.declare_dram_parameter("bqkv", [3 * MQ, 1], f32, isOutput=False)
    wpT = nc.declare_dram_parameter("wpT", [MQ, Cc], bf16, isOutput=False)
    outT = nc.declare_dram_parameter("outT", [Cc, BT], bf16, isOutput=True)

    xT_r = xT.rearrange("(kt p) t -> p kt t", p=128)
    wq_r = wqkvT.rearrange("(kt p) m -> p kt m", p=128)
    bq_r = bqkv.rearrange("(g p) o -> p (g o)", p=128)
    outT_r = outT.rearrange("(mo p) t -> p mo t", p=128)

    AF = mybir.ActivationFunctionType

    with tile.TileContext(nc) as tc:
        with (
            tc.tile_pool(name="consts", bufs=1) as consts,
            tc.tile_pool(name="xpool", bufs=3) as xpool,
            tc.tile_pool(name="spool", bufs=6) as spool,
            tc.tile_pool(name="ypool", bufs=3) as ypool,
            tc.tile_pool(name="npool", bufs=3) as npool,
            tc.tile_pool(name="opool", bufs=3) as opool,
            tc.tile_pool(name="ps_mm", bufs=4, space="PSUM") as ps_mm,
            tc.tile_pool(name="ps_y", bufs=2, space="PSUM") as ps_y,
            tc.tile_pool(name="ps_aux", bufs=2, space="PSUM") as ps_aux,
        ):
            # ---- constants; first x chunk interleaved with w for fast start
            w_sb = consts.tile([128, KT_C, 3 * MQ], bf16, tag="w")
            x_first = xpool.tile([128, KT_C, TOKC], bf16, tag="x")
            for kt in range(KT_C):
                nc.sync.dma_start(out=w_sb[:, kt, :], in_=wq_r[:, kt, :])
                nc.sync.dma_start(out=x_first[:, kt, :], in_=xT_r[:, kt, 0:TOKC])
            b_sb = consts.tile([128, 3], f32, tag="b")
            nc.sync.dma_start(out=b_sb, in_=bq_r)
            wp_sb = consts.tile([128, Cc], bf16, tag="wp")
            nc.sync.dma_start(out=wp_sb, in_=wpT[:, :])
            ident = consts.tile([128, 128], f32, tag="ident")
            make_identity(nc, ident)
            ident_bf = consts.tile([128, 128], bf16, tag="ident_bf")
            nc.vector.tensor_copy(ident_bf[:, :], ident[:, :])
            # Ltri[c,p] = -1e30 where c < p else 0 (causal-mask generator:
            # Ltri.T @ I accumulated into S's PSUM masks f < p)
            ltri = consts.tile([128, 128], bf16, tag="ltri")
            nc.gpsimd.memset(ltri[:, :], 0.0)
            nc.gpsimd.affine_select(
                out=ltri[:, :], in_=ltri[:, :],
                compare_op=mybir.AluOpType.is_ge,
                fill=-1e30,
                base=0,
                pattern=[[-1, 128]],
                channel_multiplier=1,
            )

            ones_f32 = consts.tile([128, HD], f32, tag="ones_f")
            nc.vector.memset(ones_f32[:, :], 1.0)
            ones_sb = consts.tile([1, HD], f32r, tag="ones")
            nc.scalar.activation(out=ones_sb[:, :], in_=ones_f32[0:1, :],
                                 func=AF.Copy)
            qT_sb = consts.tile([128, BT], bf16, tag="qT")
            kT_sb = consts.tile([128, BT], bf16, tag="kT")
            vT_sb = consts.tile([128, BT], bf16, tag="vT")

            # V in token-major layout: per b [128(tok), kt, 2*65] where
            # cols hh*65..hh*65+63 = V of head hh, col hh*65+64 = 1.0 (Z row)
            v_sb = [
                consts.tile([128, KTT, 2 * 65], bf16, tag=f"v{b}",
                            name=f"v{b}") for b in range(Bc)
            ]
            for b in range(Bc):
                for hh in range(hpc):
                    nc.vector.memset(v_sb[b][:, :, hh * 65 + 64:hh * 65 + 65], 1.0)

            # ---- phase 1: QKV projection (feature-major, bf16) ----
            for ch in range(NCH):
                if ch == 0:
                    x_t = x_first
                else:
                    x_t = xpool.tile([128, KT_C, TOKC], bf16, tag="x")
                    nc.sync.dma_start(
                        out=x_t, in_=xT_r[:, :, ch * TOKC:(ch + 1) * TOKC])
                for m in range(3):
                    ps = ps_mm.tile([128, TOKC], f32, tag="mm")
                    for kt in range(KT_C):
                        nc.tensor.matmul(
                            ps[:, :],
                            w_sb[:, kt, m * MQ:(m + 1) * MQ],
                            x_t[:, kt, :],
                            start=(kt == 0), stop=(kt == KT_C - 1),
                        )
                    dst = (qT_sb, kT_sb, vT_sb)[m]
                    nc.scalar.activation(
                        out=dst[:, ch * TOKC:(ch + 1) * TOKC], in_=ps[:, :],
                        func=AF.Identity, bias=b_sb[:, m:m + 1], scale=1.0,
                    )

            # ---- phase 2: V transpose -> token-major v_sb ----
            for b in range(Bc):
                for kt in range(KTT):
                    ps_t = ps_aux.tile([128, 128], bf16, tag="aux")
                    nc.tensor.transpose(
                        ps_t[:, :],
                        vT_sb[:, b * Tc + kt * 128:b * Tc + (kt + 1) * 128],
                        ident_bf[:, :],
                    )
                    for hh in range(hpc):
                        nc.vector.tensor_copy(
                            v_sb[b][:, kt, hh * 65:hh * 65 + 64],
                            ps_t[:, hh * HD:(hh + 1) * HD],
                        )

            # ---- phase 3+4: attention + projection, software-pipelined ----
            blocks = [(b, qc) for b in range(Bc) for qc in range(QC)]

            def make_norm_tail(psy, yT_t, hh):
                """lnZ now (ACT); returns emitter for bcast -> exp(-x) -> mul."""
                lnZ_t = npool.tile([1, TOKC], f32r, tag="lnz")
                nc.scalar.activation(out=lnZ_t[:, :], in_=psy[64:65, :],
                                     func=AF.Ln)

                def emit():
                    ps_bc = ps_aux.tile([HD, TOKC], f32, tag="aux")
                    nc.tensor.matmul(ps_bc[:, :], ones_sb[:, :], lnZ_t[:, :],
                                     start=True, stop=True)
                    inv_bc = npool.tile([HD, TOKC], f32, tag="invz")
                    nc.scalar.activation(out=inv_bc[:, :], in_=ps_bc[:, :],
                                         func=AF.Exp, scale=-1.0)
                    with nc.allow_low_precision(reason="yT in bf16"):
                        nc.vector.tensor_mul(
                            yT_t[hh * HD:(hh + 1) * HD, :],
                            psy[0:HD, :], inv_bc[:, :],
                        )
                return emit

            def make_proj(yT_t, b, qc):
                def emit():
                    o_t = opool.tile([128, MO, TOKC], bf16, tag="o")
                    for mo in range(MO):
                        pso = ps_mm.tile([128, TOKC], f32, tag="mm")
                        nc.tensor.matmul(
                            pso[:, :],
                            wp_sb[:, mo * 128:(mo + 1) * 128],
                            yT_t[:, :],
                            start=True, stop=True,
                        )
                        with nc.allow_low_precision(reason="partials in bf16"):
                            nc.vector.tensor_copy(o_t[:, mo, :], pso[:, :])
                        nc.sync.dma_start(
                            out=outT_r[:, mo,
                                       b * Tc + qc * TOKC:b * Tc + (qc + 1) * TOKC],
                            in_=o_t[:, mo, :],
                        )
                return emit

            # deferred work queue: emitters injected into later PE streams
            pending = []

            def emit_head(b, qc, hh, yT_t):
                n_kt = (qc + 1) * DTILE
                q0 = b * Tc + qc * TOKC
                psy = ps_y.tile([65, TOKC], f32, tag="y")
                exp_tiles = {}

                def emit_S(kt):
                    di = kt - qc * DTILE
                    c0 = max(0, di) * 128       # local column start
                    W = TOKC - c0
                    pss = ps_mm.tile([128, TOKC], f32, tag="mm")
                    nc.tensor.matmul(
                        pss[:, 0:W],
                        kT_sb[hh * HD:(hh + 1) * HD,
                              b * Tc + kt * 128:b * Tc + (kt + 1) * 128],
                        qT_sb[hh * HD:(hh + 1) * HD, q0 + c0:q0 + TOKC],
                        start=True, stop=(di < 0),
                    )
                    if di >= 0:
                        # causal mask: += Ltri.T @ I over the leading 128
                        # cols (-1e30 where f_local < p)
                        nc.tensor.matmul(
                            pss[:, 0:128],
                            ltri[:, :],
                            ident_bf[:, :],
                            start=False, stop=True,
                            skip_group_check=True,
                        )
                    e_t = spool.tile([128, TOKC], bf16, tag="e")
                    nc.scalar.activation(out=e_t[:, 0:W], in_=pss[:, 0:W],
                                         func=AF.Exp, scale=0.125)
                    exp_tiles[kt] = (e_t, c0, W)

                def emit_AV(kt):
                    e_t, c0, W = exp_tiles.pop(kt)
                    nc.tensor.matmul(
                        psy[:, c0:TOKC],
                        v_sb[b][:, kt, hh * 65:(hh + 1) * 65],
                        e_t[:, 0:W],
                        start=(kt == 0), stop=(kt == n_kt - 1),
                    )

                DEPTH = 3
                for kt in range(n_kt):
                    emit_S(kt)
                    # drain deferred norm/proj work into this PE stream
                    if kt == 2 and pending:
                        pending.pop(0)()
                    elif kt == 4 and pending:
                        pending.pop(0)()
                    if kt >= DEPTH:
                        emit_AV(kt - DEPTH)
                for kt in range(max(0, n_kt - DEPTH), n_kt):
                    emit_AV(kt)
                pending.append(make_norm_tail(psy, yT_t, hh))

            prev = None
            for b, qc in blocks:
                yT_t = ypool.tile([128, TOKC], bf16, tag="yT")
                emit_head(b, qc, 0, yT_t)
                if prev is not None:
                    pending.append(make_proj(*prev))
                emit_head(b, qc, 1, yT_t)
                prev = (yT_t, b, qc)
            pending.append(make_proj(*prev))
            while pending:
                pending.pop(0)()

    nc.finalize()
    return nc


def prep_inputs(cfg, x, W_attn, b_attn, W_proj, b_proj):
    """Host-side sharding: returns per-core input dicts."""
    import ml_dtypes
    Bc, Tc, Cc, hpc = cfg["B"], cfg["T"], cfg["C"], cfg["HPC"]
    n_cores = (Cc // HD) // hpc
    BT = Bc * Tc
    MQ = hpc * HD

    x = np.ascontiguousarray(x, dtype=np.float32)
    xT = np.ascontiguousarray(x.reshape(BT, Cc).T).astype(ml_dtypes.bfloat16)

    in_maps = []
    for c in range(n_cores):
        r0 = c * MQ
        rows = []
        for g in range(3):
            rows.append(np.arange(g * Cc + r0, g * Cc + r0 + MQ))
        rows = np.concatenate(rows)
        w_slice = W_attn[rows, :]                       # [384, C]
        wqkvT = np.ascontiguousarray(w_slice.T).astype(ml_dtypes.bfloat16)
        bq = np.ascontiguousarray(b_attn[rows].reshape(MQ * 3, 1))
        wpT = np.ascontiguousarray(W_proj[:, r0:r0 + MQ].T).astype(ml_dtypes.bfloat16)
        in_maps.append({
            "xT": xT,
            "wqkvT": wqkvT,
            "bqkv": bq.astype(np.float32),
            "wpT": wpT,
        })
    return in_maps


def combine(cfg, results, b_proj):
    Bc, Tc, Cc = cfg["B"], cfg["T"], cfg["C"]
    acc = results[0]["outT"].astype(np.float32)
    for r in results[1:]:
        acc += r["outT"].astype(np.float32)
    out = acc.T + b_proj[None, :]
    return np.ascontiguousarray(out.reshape(Bc, Tc, Cc).astype(np.float32))


_NC_CACHE = {}


def kernel(x, W_attn, b_attn, W_proj, b_proj):
    from concourse.bass_utils import run_bass_kernel_spmd

    cfg = _cfg_full()
    key = "full"
    if key not in _NC_CACHE:
        _NC_CACHE[key] = build_nc(cfg)
    nc = _NC_CACHE[key]
    in_maps = prep_inputs(cfg, np.asarray(x), np.asarray(W_attn),
                          np.asarray(b_attn), np.asarray(W_proj),
                          np.asarray(b_proj))
    res = run_bass_kernel_spmd(nc, in_maps, list(range(N_CORES)))
    return combine(cfg, res.results, np.asarray(b_proj, dtype=np.float32))


# revision 9
# speedup vs baseline: 1.5759x; 1.0034x over previous
"""Causal self-attention Trainium2 kernel (8-core head-parallel tensor parallel).

Strategy (v3):
  - 16 heads split across 8 cores (2 heads each), all-bf16 PE dataflow.
  - qkv^T = W^T.T @ x^T (bf16), bias added during ACT eviction.
  - Attention per (b, head, q-chunk of 512), feature-major:
      S^T[k,q] = K^T.T @ Q^T  (bf16; diagonal tiles column-trimmed).
      Causal mask: extra PE matmul accumulating Ltri(-1e30) @ I into the
      S PSUM group (no DVE/gpsimd in the dependency chain).
      expS = exp(0.125 * S^T) on ACT -> bf16.
      [y^T; Z] = [V | 1]^T.T @ expS  (PE accumulate; row 64 = Z).
      1/Z = exp(-ln(Z)) on ACT; all activation funcs pinned to the
      natural_log_exp_and_others table set (one ACT_TABLE_LOAD total).
      y^T = psy * bcast(1/Z) (DVE).
  - Normalize + projection emission deferred into later PE streams so the
    PE never idles (keeps the HAM clock warm).
  - out^T partial = Wp^T.T @ y^T -> bf16 partials -> DRAM; host sums.
"""

import sys

if "/opt/trn_rl_repo" not in sys.path:
    sys.path.insert(0, "/opt/trn_rl_repo")

import numpy as np

# ---- problem constants (hardcoded for the grading harness) ----
B, T, C, H = 2, 2048, 1024, 16
HD = C // H            # 64
N_CORES = 8
HPC = H // N_CORES     # heads per core = 2


def _cfg_full():
    return dict(B=B, T=T, C=C, HPC=HPC)


def _make_bacc():
    """Bacc with all activation funcs pinned to the one table set that
    contains both exp and ln, so no ACT_TABLE_LOAD thrash."""
    import concourse.bacc as bacc
    import bass_rust as _bass_rust
    from concourse.hw_specs import get_activation_tables

    class BaccPinnedAct(bacc.Bacc):
        def insert_act_table_loads(self):
            tables = list(get_activation_tables(self.m.arch).items())
            doctored = []
            for name, fns in tables:
                if name == "natural_log_exp_and_others":
                    doctored.append((name, fns))
                else:
                    doctored.append((name, set()))
            _bass_rust.insert_act_table_loads(self, doctored)

    return BaccPinnedAct()


def build_nc(cfg):
    """Build the single-core SPMD Bass program."""
    import concourse.mybir as mybir
    import concourse.tile as tile
    from concourse.masks import make_identity

    Bc, Tc, Cc, hpc = cfg["B"], cfg["T"], cfg["C"], cfg["HPC"]
    f32r = mybir.dt.float32r
    f32 = mybir.dt.float32
    bf16 = mybir.dt.bfloat16
    BT = Bc * Tc
    MQ = hpc * HD                 # rows per m-group (q|k|v) = 128
    assert MQ == 128
    KT_C = Cc // 128              # contraction tiles for qkv/x
    TOKC = 512
    NCH = BT // TOKC              # token chunks over both batches
    QC = Tc // TOKC               # q-chunks per batch
    KTT = Tc // 128               # k-tiles per batch
    MO = Cc // 128                # proj output tiles
    DTILE = TOKC // 128           # 4 diagonal tiles per q-chunk

    nc = _make_bacc()
    xT = nc.declare_dram_parameter("xT", [Cc, BT], bf16, isOutput=False)
    wqkvT = nc.declare_dram_parameter("wqkvT", [Cc, 3 * MQ], bf16, isOutput=False)
    bqkv = nc.declare_dram_parameter("bqkv", [3 * MQ, 1], f32, isOutput=False)
    wpT = nc.declare_dram_parameter("wpT", [MQ, Cc], bf16, isOutput=False)
    outT = nc.declare_dram_parameter("outT", [Cc, BT], bf16, isOutput=True)

    xT_r = xT.rearrange("(kt p) t -> p kt t", p=128)
    wq_r = wqkvT.rearrange("(kt p) m -> p kt m", p=128)
    bq_r = bqkv.rearrange("(g p) o -> p (g o)", p=128)
    outT_r = outT.rearrange("(mo p) t -> p mo t", p=128)

    AF = mybir.ActivationFunctionType

    with tile.TileContext(nc) as tc:
        with (
            tc.tile_pool(name="consts", bufs=1) as consts,
            tc.tile_pool(name="xpool", bufs=3) as xpool,
            tc.tile_pool(name="spool", bufs=6) as spool,
            tc.tile_pool(name="ypool", bufs=3) as ypool,
            tc.tile_pool(name="npool", bufs=3) as npool,
            tc.tile_pool(name="opool", bufs=3) as opool,
            tc.tile_pool(name="ps_mm", bufs=4, space="PSUM") as ps_mm,
            tc.tile_pool(name="ps_y", bufs=2, space="PSUM") as ps_y,
            tc.tile_pool(name="ps_aux", bufs=2, space="PSUM") as ps_aux,
        ):
            # ---- constants; w on sync queue, x on act queue (parallel issue)
            w_sb = consts.tile([128, KT_C, 3 * MQ], bf16, tag="w")
            x_first = xpool.tile([128, KT_C, TOKC], bf16, tag="x")
            for kt in range(KT_C):
                nc.sync.dma_start(out=w_sb[:, kt, :], in_=wq_r[:, kt, :])
                nc.scalar.dma_start(out=x_first[:, kt, :], in_=xT_r[:, kt, 0:TOKC])
            b_sb = consts.tile([128, 3], f32, tag="b")
            nc.sync.dma_start(out=b_sb, in_=bq_r)
            wp_sb = consts.tile([128, Cc], bf16, tag="wp")
            nc.sync.dma_start(out=wp_sb, in_=wpT[:, :])
            ident = consts.tile([128, 128], f32, tag="ident")
            make_identity(nc, ident)
            ident_bf = consts.tile([128, 128], bf16, tag="ident_bf")
            nc.vector.tensor_copy(ident_bf[:, :], ident[:, :])
            # Ltri[c,p] = -1e30 where c < p else 0 (causal-mask generator:
            # Ltri.T @ I accumulated into S's PSUM masks f < p)
            ltri = consts.tile([128, 128], bf16, tag="ltri")
            nc.gpsimd.memset(ltri[:, :], 0.0)
            nc.gpsimd.affine_select(
                out=ltri[:, :], in_=ltri[:, :],
                compare_op=mybir.AluOpType.is_ge,
                fill=-1e30,
                base=0,
                pattern=[[-1, 128]],
                channel_multiplier=1,
            )

            ones_f32 = consts.tile([128, HD], f32, tag="ones_f")
            nc.vector.memset(ones_f32[:, :], 1.0)
            ones_sb = consts.tile([1, HD], f32r, tag="ones")
            nc.scalar.activation(out=ones_sb[:, :], in_=ones_f32[0:1, :],
                                 func=AF.Copy)
            qT_sb = consts.tile([128, BT], bf16, tag="qT")
            kT_sb = consts.tile([128, BT], bf16, tag="kT")
            vT_sb = consts.tile([128, BT], bf16, tag="vT")

            # V in token-major layout: per b [128(tok), kt, 2*65] where
            # cols hh*65..hh*65+63 = V of head hh, col hh*65+64 = 1.0 (Z row)
            v_sb = [
                consts.tile([128, KTT, 2 * 65], bf16, tag=f"v{b}",
                            name=f"v{b}") for b in range(Bc)
            ]
            for b in range(Bc):
                for hh in range(hpc):
                    nc.vector.memset(v_sb[b][:, :, hh * 65 + 64:hh * 65 + 65], 1.0)

            # ---- phase 1: QKV projection (feature-major, bf16) ----
            for ch in range(NCH):
                if ch == 0:
                    x_t = x_first
                else:
                    x_t = xpool.tile([128, KT_C, TOKC], bf16, tag="x")
                    nc.scalar.dma_start(
                        out=x_t, in_=xT_r[:, :, ch * TOKC:(ch + 1) * TOKC])
                for m in range(3):
                    ps = ps_mm.tile([128, TOKC], f32, tag="mm")
                    for kt in range(KT_C):
                        nc.tensor.matmul(
                            ps[:, :],
                            w_sb[:, kt, m * MQ:(m + 1) * MQ],
                            x_t[:, kt, :],
                            start=(kt == 0), stop=(kt == KT_C - 1),
                        )
                    dst = (qT_sb, kT_sb, vT_sb)[m]
                    nc.scalar.activation(
                        out=dst[:, ch * TOKC:(ch + 1) * TOKC], in_=ps[:, :],
                        func=AF.Identity, bias=b_sb[:, m:m + 1], scale=1.0,
                    )

            # ---- phase 2: V transpose -> token-major v_sb ----
            for b in range(Bc):
                for kt in range(KTT):
                    ps_t = ps_aux.tile([128, 128], bf16, tag="aux")
                    nc.tensor.transpose(
                        ps_t[:, :],
                        vT_sb[:, b * Tc + kt * 128:b * Tc + (kt + 1) * 128],
                        ident_bf[:, :],
                    )
                    for hh in range(hpc):
                        nc.vector.tensor_copy(
                            v_sb[b][:, kt, hh * 65:hh * 65 + 64],
                            ps_t[:, hh * HD:(hh + 1) * HD],
                        )

            # ---- phase 3+4: attention + projection, software-pipelined ----
            blocks = [(b, qc) for b in range(Bc) for qc in range(QC)]

            def make_norm_tail(psy, yT_t, hh):
                """lnZ now (ACT); returns emitter for bcast -> exp(-x) -> mul."""
                lnZ_t = npool.tile([1, TOKC], f32r, tag="lnz")
                nc.scalar.activation(out=lnZ_t[:, :], in_=psy[64:65, :],
                                     func=AF.Ln)

                def emit():
                    ps_bc = ps_aux.tile([HD, TOKC], f32, tag="aux")
                    nc.tensor.matmul(ps_bc[:, :], ones_sb[:, :], lnZ_t[:, :],
                                     start=True, stop=True)
                    inv_bc = npool.tile([HD, TOKC], f32, tag="invz")
                    nc.scalar.activation(out=inv_bc[:, :], in_=ps_bc[:, :],
                                         func=AF.Exp, scale=-1.0)
                    with nc.allow_low_precision(reason="yT in bf16"):
                        nc.vector.tensor_mul(
                            yT_t[hh * HD:(hh + 1) * HD, :],
                            psy[0:HD, :], inv_bc[:, :],
                        )
                return emit

            def make_proj(yT_t, b, qc, last=False):
                def emit():
                    o_t = opool.tile([128, MO, TOKC], bf16, tag="o")
                    for mo in range(MO):
                        pso = ps_mm.tile([128, TOKC], f32, tag="mm")
                        nc.tensor.matmul(
                            pso[:, :],
                            wp_sb[:, mo * 128:(mo + 1) * 128],
                            yT_t[:, :],
                            start=True, stop=True,
                        )
                        with nc.allow_low_precision(reason="partials in bf16"):
                            if last and mo % 2 == 1:
                                nc.scalar.activation(
                                    out=o_t[:, mo, :], in_=pso[:, :],
                                    func=AF.Copy)
                            else:
                                nc.vector.tensor_copy(o_t[:, mo, :], pso[:, :])
                        nc.sync.dma_start(
                            out=outT_r[:, mo,
                                       b * Tc + qc * TOKC:b * Tc + (qc + 1) * TOKC],
                            in_=o_t[:, mo, :],
                        )
                return emit

            # deferred work queue: emitters injected into later PE streams
            pending = []

            def emit_head(b, qc, hh, yT_t):
                n_kt = (qc + 1) * DTILE
                q0 = b * Tc + qc * TOKC
                psy = ps_y.tile([65, TOKC], f32, tag="y")
                exp_tiles = {}

                def geom(kt):
                    di = kt - qc * DTILE
                    c0 = max(0, di) * 128       # local column start
                    return di, c0, TOKC - c0

                def emit_S(kt):
                    di, c0, W = geom(kt)
                    pss = ps_mm.tile([128, TOKC], f32, tag="mm")
                    nc.tensor.matmul(
                        pss[:, 0:W],
                        kT_sb[hh * HD:(hh + 1) * HD,
                              b * Tc + kt * 128:b * Tc + (kt + 1) * 128],
                        qT_sb[hh * HD:(hh + 1) * HD, q0 + c0:q0 + TOKC],
                        start=True, stop=(di < 0),
                    )
                    exp_tiles[kt] = (pss, None, c0, W)

                def emit_mask(kt):
                    di, c0, W = geom(kt)
                    if di < 0:
                        return
                    pss = exp_tiles[kt][0]
                    # causal mask: += Ltri.T @ I over the leading 128 cols
                    # (-1e30 where f_local < p)
                    nc.tensor.matmul(
                        pss[:, 0:128],
                        ltri[:, :],
                        ident_bf[:, :],
                        start=False, stop=True,
                        skip_group_check=True,
                    )

                def emit_exp(kt):
                    pss, _, c0, W = exp_tiles[kt]
                    e_t = spool.tile([128, TOKC], bf16, tag="e")
                    nc.scalar.activation(out=e_t[:, 0:W], in_=pss[:, 0:W],
                                         func=AF.Exp, scale=0.125)
                    exp_tiles[kt] = (pss, e_t, c0, W)

                def emit_AV(kt):
                    _, e_t, c0, W = exp_tiles.pop(kt)
                    nc.tensor.matmul(
                        psy[:, c0:TOKC],
                        v_sb[b][:, kt, hh * 65:(hh + 1) * 65],
                        e_t[:, 0:W],
                        start=(kt == 0), stop=(kt == n_kt - 1),
                    )

                # groups of 2 k-tiles: [S,S] in 64-row mode, then
                # [mask,mask,AV(prev),AV(prev)] in 128-row mode — minimizes
                # PE tiling-mode switches (each switch drains the array)
                GS = 2
                groups = [list(range(g, min(g + GS, n_kt)))
                          for g in range(0, n_kt, GS)]
                prev_g = None
                for gi, g in enumerate(groups):
                    for kt in g:
                        emit_S(kt)
                    for kt in g:
                        emit_mask(kt)
                    for kt in g:
                        emit_exp(kt)
                    if prev_g is not None:
                        for kt in prev_g:
                            emit_AV(kt)
                    # drain deferred norm/proj work into this PE stream
                    if gi in (1, 2) and pending:
                        pending.pop(0)()
                    prev_g = g
                for kt in prev_g:
                    emit_AV(kt)
                pending.append(make_norm_tail(psy, yT_t, hh))

            prev = None
            for b, qc in blocks:
                yT_t = ypool.tile([128, TOKC], bf16, tag="yT")
                emit_head(b, qc, 0, yT_t)
                if prev is not None:
                    pending.append(make_proj(*prev))
                emit_head(b, qc, 1, yT_t)
                prev = (yT_t, b, qc)
            pending.append(make_proj(*prev, last=True))
            while pending:
                pending.pop(0)()

    nc.finalize()
    return nc


def prep_inputs(cfg, x, W_attn, b_attn, W_proj, b_proj):
    """Host-side sharding: returns per-core input dicts."""
    import ml_dtypes
    Bc, Tc, Cc, hpc = cfg["B"], cfg["T"], cfg["C"], cfg["HPC"]
    n_cores = (Cc // HD) // hpc
    BT = Bc * Tc
    MQ = hpc * HD

    x = np.ascontiguousarray(x, dtype=np.float32)
    xT = np.ascontiguousarray(x.reshape(BT, Cc).T).astype(ml_dtypes.bfloat16)

    in_maps = []
    for c in range(n_cores):
        r0 = c * MQ
        rows = []
        for g in range(3):
            rows.append(np.arange(g * Cc + r0, g * Cc + r0 + MQ))
        rows = np.concatenate(rows)
        w_slice = W_attn[rows, :]                       # [384, C]
        wqkvT = np.ascontiguousarray(w_slice.T).astype(ml_dtypes.bfloat16)
        bq = np.ascontiguousarray(b_attn[rows].reshape(MQ * 3, 1))
        wpT = np.ascontiguousarray(W_proj[:, r0:r0 + MQ].T).astype(ml_dtypes.bfloat16)
        in_maps.append({
            "xT": xT,
            "wqkvT": wqkvT,
            "bqkv": bq.astype(np.float32),
            "wpT": wpT,
        })
    return in_maps


def combine(cfg, results, b_proj):
    Bc, Tc, Cc = cfg["B"], cfg["T"], cfg["C"]
    acc = results[0]["outT"].astype(np.float32)
    for r in results[1:]:
        acc += r["outT"].astype(np.float32)
    out = acc.T + b_proj[None, :]
    return np.ascontiguousarray(out.reshape(Bc, Tc, Cc).astype(np.float32))


_NC_CACHE = {}


def kernel(x, W_attn, b_attn, W_proj, b_proj):
    from concourse.bass_utils import run_bass_kernel_spmd

    cfg = _cfg_full()
    key = "full"
    if key not in _NC_CACHE:
        _NC_CACHE[key] = build_nc(cfg)
    nc = _NC_CACHE[key]
    in_maps = prep_inputs(cfg, np.asarray(x), np.asarray(W_attn),
                          np.asarray(b_attn), np.asarray(W_proj),
                          np.asarray(b_proj))
    res = run_bass_kernel_spmd(nc, in_maps, list(range(N_CORES)))
    return combine(cfg, res.results, np.asarray(b_proj, dtype=np.float32))


# revision 12
# speedup vs baseline: 1.6165x; 1.0257x over previous
"""Causal self-attention Trainium2 kernel (8-core head-parallel tensor parallel).

Strategy (v3):
  - 16 heads split across 8 cores (2 heads each), all-bf16 PE dataflow.
  - qkv^T = W^T.T @ x^T (bf16), bias added during ACT eviction.
  - Attention per (b, head, q-chunk of 512), feature-major:
      S^T[k,q] = K^T.T @ Q^T  (bf16; diagonal tiles column-trimmed).
      Causal mask: extra PE matmul accumulating Ltri(-1e30) @ I into the
      S PSUM group (no DVE/gpsimd in the dependency chain).
      expS = exp(0.125 * S^T) on ACT -> bf16.
      [y^T; Z] = [V | 1]^T.T @ expS  (PE accumulate; row 64 = Z).
      1/Z = exp(-ln(Z)) on ACT; all activation funcs pinned to the
      natural_log_exp_and_others table set (one ACT_TABLE_LOAD total).
      y^T = psy * bcast(1/Z) (DVE).
  - Normalize + projection emission deferred into later PE streams so the
    PE never idles (keeps the HAM clock warm).
  - out^T partial = Wp^T.T @ y^T -> bf16 partials -> DRAM; host sums.
"""

import sys

if "/opt/trn_rl_repo" not in sys.path:
    sys.path.insert(0, "/opt/trn_rl_repo")

import numpy as np

# ---- problem constants (hardcoded for the grading harness) ----
B, T, C, H = 2, 2048, 1024, 16
HD = C // H            # 64
N_CORES = 8
HPC = H // N_CORES     # heads per core = 2


def _cfg_full():
    return dict(B=B, T=T, C=C, HPC=HPC)


def _make_bacc():
    """Bacc with all activation funcs pinned to the one table set that
    contains both exp and ln, so no ACT_TABLE_LOAD thrash."""
    import concourse.bacc as bacc
    import bass_rust as _bass_rust
    from concourse.hw_specs import get_activation_tables

    class BaccPinnedAct(bacc.Bacc):
        def insert_act_table_loads(self):
            tables = list(get_activation_tables(self.m.arch).items())
            doctored = []
            for name, fns in tables:
                if name == "natural_log_exp_and_others":
                    doctored.append((name, fns))
                else:
                    doctored.append((name, set()))
            _bass_rust.insert_act_table_loads(self, doctored)

    return BaccPinnedAct()


def build_nc(cfg):
    """Build the single-core SPMD Bass program."""
    import concourse.mybir as mybir
    import concourse.tile as tile
    from concourse.masks import make_identity

    Bc, Tc, Cc, hpc = cfg["B"], cfg["T"], cfg["C"], cfg["HPC"]
    f32r = mybir.dt.float32r
    f32 = mybir.dt.float32
    bf16 = mybir.dt.bfloat16
    BT = Bc * Tc
    MQ = hpc * HD                 # rows per m-group (q|k|v) = 128
    assert MQ == 128
    KT_C = Cc // 128              # contraction tiles for qkv/x
    TOKC = 512
    NCH = BT // TOKC              # token chunks over both batches
    QC = Tc // TOKC               # q-chunks per batch
    KTT = Tc // 128               # k-tiles per batch
    MO = Cc // 128                # proj output tiles
    DTILE = TOKC // 128           # 4 diagonal tiles per q-chunk

    nc = _make_bacc()
    xT = nc.declare_dram_parameter("xT", [Cc, BT], bf16, isOutput=False)
    wqkvT = nc.declare_dram_parameter("wqkvT", [Cc, 3 * MQ], bf16, isOutput=False)
    bqkv = nc.declare_dram_parameter("bqkv", [3 * MQ, 1], f32, isOutput=False)
    wpT = nc.declare_dram_parameter("wpT", [MQ, Cc], bf16, isOutput=False)
    outT = nc.declare_dram_parameter("outT", [Cc, BT], bf16, isOutput=True)

    xT_r = xT.rearrange("(kt p) t -> p kt t", p=128)
    wq_r = wqkvT.rearrange("(kt p) m -> p kt m", p=128)
    bq_r = bqkv.rearrange("(g p) o -> p (g o)", p=128)
    outT_r = outT.rearrange("(mo p) t -> p mo t", p=128)

    AF = mybir.ActivationFunctionType

    with tile.TileContext(nc) as tc:
        with (
            tc.tile_pool(name="consts", bufs=1) as consts,
            tc.tile_pool(name="xpool", bufs=3) as xpool,
            tc.tile_pool(name="spool", bufs=10) as spool,
            tc.tile_pool(name="ypool", bufs=3) as ypool,
            tc.tile_pool(name="npool", bufs=3) as npool,
            tc.tile_pool(name="opool", bufs=3) as opool,
            tc.tile_pool(name="ps_mm", bufs=4, space="PSUM") as ps_mm,
            tc.tile_pool(name="ps_y", bufs=2, space="PSUM") as ps_y,
            tc.tile_pool(name="ps_aux", bufs=2, space="PSUM") as ps_aux,
        ):
            # ---- constants; w on sync queue, x on act queue (parallel issue)
            w_sb = consts.tile([128, KT_C, 3 * MQ], bf16, tag="w")
            x_first = xpool.tile([128, KT_C, TOKC], bf16, tag="x")
            for kt in range(KT_C):
                nc.sync.dma_start(out=w_sb[:, kt, :], in_=wq_r[:, kt, :])
                nc.scalar.dma_start(out=x_first[:, kt, :], in_=xT_r[:, kt, 0:TOKC])
            b_sb = consts.tile([128, 3], f32, tag="b")
            nc.sync.dma_start(out=b_sb, in_=bq_r)
            wp_sb = consts.tile([128, Cc], bf16, tag="wp")
            nc.sync.dma_start(out=wp_sb, in_=wpT[:, :])
            ident = consts.tile([128, 128], f32, tag="ident")
            make_identity(nc, ident)
            ident_bf = consts.tile([128, 128], bf16, tag="ident_bf")
            nc.vector.tensor_copy(ident_bf[:, :], ident[:, :])
            # step[p,f] = 1.0 where f >= p else 0 (multiplicative causal
            # mask applied post-exp on DVE over the diagonal 128 cols)
            step = consts.tile([128, 128], bf16, tag="step")
            nc.gpsimd.memset(step[:, :], 1.0)
            nc.gpsimd.affine_select(
                out=step[:, :], in_=step[:, :],
                compare_op=mybir.AluOpType.is_ge,
                fill=0.0,
                base=0,
                pattern=[[1, 128]],
                channel_multiplier=-1,
            )

            ones_f32 = consts.tile([128, HD], f32, tag="ones_f")
            nc.vector.memset(ones_f32[:, :], 1.0)
            ones_sb = consts.tile([1, HD], f32r, tag="ones")
            nc.scalar.activation(out=ones_sb[:, :], in_=ones_f32[0:1, :],
                                 func=AF.Copy)
            qT_sb = consts.tile([128, BT], bf16, tag="qT")
            kT_sb = consts.tile([128, BT], bf16, tag="kT")
            vT_sb = consts.tile([128, BT], bf16, tag="vT")

            # V in token-major layout: per b [128(tok), kt, 2*65] where
            # cols hh*65..hh*65+63 = V of head hh, col hh*65+64 = 1.0 (Z row)
            v_sb = [
                consts.tile([128, KTT, 2 * 65], bf16, tag=f"v{b}",
                            name=f"v{b}") for b in range(Bc)
            ]
            for b in range(Bc):
                for hh in range(hpc):
                    nc.vector.memset(v_sb[b][:, :, hh * 65 + 64:hh * 65 + 65], 1.0)

            # ---- phase 1: QKV projection (feature-major, bf16) ----
            for ch in range(NCH):
                if ch == 0:
                    x_t = x_first
                else:
                    x_t = xpool.tile([128, KT_C, TOKC], bf16, tag="x")
                    nc.scalar.dma_start(
                        out=x_t, in_=xT_r[:, :, ch * TOKC:(ch + 1) * TOKC])
                for m in range(3):
                    ps = ps_mm.tile([128, TOKC], f32, tag="mm")
                    for kt in range(KT_C):
                        nc.tensor.matmul(
                            ps[:, :],
                            w_sb[:, kt, m * MQ:(m + 1) * MQ],
                            x_t[:, kt, :],
                            start=(kt == 0), stop=(kt == KT_C - 1),
                        )
                    dst = (qT_sb, kT_sb, vT_sb)[m]
                    nc.scalar.activation(
                        out=dst[:, ch * TOKC:(ch + 1) * TOKC], in_=ps[:, :],
                        func=AF.Identity, bias=b_sb[:, m:m + 1], scale=1.0,
                    )

            # ---- phase 2: V transpose -> token-major v_sb ----
            for b in range(Bc):
                for kt in range(KTT):
                    ps_t = ps_aux.tile([128, 128], bf16, tag="aux")
                    nc.tensor.transpose(
                        ps_t[:, :],
                        vT_sb[:, b * Tc + kt * 128:b * Tc + (kt + 1) * 128],
                        ident_bf[:, :],
                    )
                    for hh in range(hpc):
                        nc.vector.tensor_copy(
                            v_sb[b][:, kt, hh * 65:hh * 65 + 64],
                            ps_t[:, hh * HD:(hh + 1) * HD],
                        )

            # ---- phase 3+4: attention + projection, software-pipelined ----
            blocks = [(b, qc) for b in range(Bc) for qc in range(QC)]

            def make_norm_tail(psy, yT_t, hh):
                """lnZ now (ACT); returns emitter for bcast -> exp(-x) -> mul."""
                lnZ_t = npool.tile([1, TOKC], f32r, tag="lnz")
                nc.scalar.activation(out=lnZ_t[:, :], in_=psy[64:65, :],
                                     func=AF.Ln)

                def emit():
                    ps_bc = ps_aux.tile([HD, TOKC], f32, tag="aux")
                    nc.tensor.matmul(ps_bc[:, :], ones_sb[:, :], lnZ_t[:, :],
                                     start=True, stop=True)
                    inv_bc = npool.tile([HD, TOKC], f32, tag="invz")
                    nc.scalar.activation(out=inv_bc[:, :], in_=ps_bc[:, :],
                                         func=AF.Exp, scale=-1.0)
                    with nc.allow_low_precision(reason="yT in bf16"):
                        nc.vector.tensor_mul(
                            yT_t[hh * HD:(hh + 1) * HD, :],
                            psy[0:HD, :], inv_bc[:, :],
                        )
                return emit

            def make_proj(yT_t, b, qc, last=False):
                def emit():
                    o_t = opool.tile([128, MO, TOKC], bf16, tag="o")
                    for mo in range(MO):
                        pso = ps_mm.tile([128, TOKC], f32, tag="mm")
                        nc.tensor.matmul(
                            pso[:, :],
                            wp_sb[:, mo * 128:(mo + 1) * 128],
                            yT_t[:, :],
                            start=True, stop=True,
                        )
                        with nc.allow_low_precision(reason="partials in bf16"):
                            if last and mo % 2 == 1:
                                nc.scalar.activation(
                                    out=o_t[:, mo, :], in_=pso[:, :],
                                    func=AF.Copy)
                            else:
                                nc.vector.tensor_copy(o_t[:, mo, :], pso[:, :])
                        nc.sync.dma_start(
                            out=outT_r[:, mo,
                                       b * Tc + qc * TOKC:b * Tc + (qc + 1) * TOKC],
                            in_=o_t[:, mo, :],
                        )
                return emit

            # deferred work queue: emitters injected into later PE streams
            pending = []

            def emit_head(b, qc, hh, yT_t):
                n_kt = (qc + 1) * DTILE
                q0 = b * Tc + qc * TOKC
                psy = ps_y.tile([65, TOKC], f32, tag="y")
                exp_tiles = {}

                def geom(kt):
                    di = kt - qc * DTILE
                    c0 = max(0, di) * 128       # local column start
                    return di, c0, TOKC - c0

                def emit_S(kt):
                    di, c0, W = geom(kt)
                    pss = ps_mm.tile([128, TOKC], f32, tag="mm")
                    nc.tensor.matmul(
                        pss[:, 0:W],
                        kT_sb[hh * HD:(hh + 1) * HD,
                              b * Tc + kt * 128:b * Tc + (kt + 1) * 128],
                        qT_sb[hh * HD:(hh + 1) * HD, q0 + c0:q0 + TOKC],
                        start=True, stop=True,
                    )
                    # exp right away (frees the PSUM tile asap); causal
                    # zeroing post-exp on DVE (multiplicative step mask)
                    e_t = spool.tile([128, TOKC], bf16, tag="e")
                    nc.scalar.activation(out=e_t[:, 0:W], in_=pss[:, 0:W],
                                         func=AF.Exp, scale=0.125)
                    if di >= 0:
                        nc.vector.tensor_mul(e_t[:, 0:128], e_t[:, 0:128],
                                             step[:, :])
                    exp_tiles[kt] = (e_t, c0, W)

                def emit_AV(kt):
                    e_t, c0, W = exp_tiles.pop(kt)
                    nc.tensor.matmul(
                        psy[:, c0:TOKC],
                        v_sb[b][:, kt, hh * 65:(hh + 1) * 65],
                        e_t[:, 0:W],
                        start=(kt == 0), stop=(kt == n_kt - 1),
                    )

                # groups of 2 k-tiles: [S,S] in 64-row mode, then AVs from
                # two groups back in 128-row mode — few tiling-mode switches
                # and ~2us of exp->AV lookahead so AVs never wait on ACT.
                GS = 2
                groups = [list(range(g, min(g + GS, n_kt)))
                          for g in range(0, n_kt, GS)]
                for gi, g in enumerate(groups):
                    for kt in g:
                        emit_S(kt)
                    if gi >= 2:
                        for kt in groups[gi - 2]:
                            emit_AV(kt)
                    # drain deferred norm/proj work into this PE stream
                    if gi in (1, 2) and pending:
                        pending.pop(0)()
                for g in groups[-2:] if len(groups) >= 2 else groups:
                    for kt in g:
                        emit_AV(kt)
                pending.append(make_norm_tail(psy, yT_t, hh))

            prev = None
            for b, qc in blocks:
                yT_t = ypool.tile([128, TOKC], bf16, tag="yT")
                emit_head(b, qc, 0, yT_t)
                if prev is not None:
                    pending.append(make_proj(*prev))
                emit_head(b, qc, 1, yT_t)
                prev = (yT_t, b, qc)
            pending.append(make_proj(*prev, last=True))
            while pending:
                pending.pop(0)()

    nc.finalize()
    return nc


def prep_inputs(cfg, x, W_attn, b_attn, W_proj, b_proj):
    """Host-side sharding: returns per-core input dicts."""
    import ml_dtypes
    Bc, Tc, Cc, hpc = cfg["B"], cfg["T"], cfg["C"], cfg["HPC"]
    n_cores = (Cc // HD) // hpc
    BT = Bc * Tc
    MQ = hpc * HD

    x = np.ascontiguousarray(x, dtype=np.float32)
    xT = np.ascontiguousarray(x.reshape(BT, Cc).T).astype(ml_dtypes.bfloat16)

    in_maps = []
    for c in range(n_cores):
        r0 = c * MQ
        rows = []
        for g in range(3):
            rows.append(np.arange(g * Cc + r0, g * Cc + r0 + MQ))
        rows = np.concatenate(rows)
        w_slice = W_attn[rows, :]                       # [384, C]
        wqkvT = np.ascontiguousarray(w_slice.T).astype(ml_dtypes.bfloat16)
        bq = np.ascontiguousarray(b_attn[rows].reshape(MQ * 3, 1))
        wpT = np.ascontiguousarray(W_proj[:, r0:r0 + MQ].T).astype(ml_dtypes.bfloat16)
        in_maps.append({
            "xT": xT,
            "wqkvT": wqkvT,
            "bqkv": bq.astype(np.float32),
            "wpT": wpT,
        })
    return in_maps


def combine(cfg, results, b_proj):
    Bc, Tc, Cc = cfg["B"], cfg["T"], cfg["C"]
    acc = results[0]["outT"].astype(np.float32)
    for r in results[1:]:
        acc += r["outT"].astype(np.float32)
    out = acc.T + b_proj[None, :]
    return np.ascontiguousarray(out.reshape(Bc, Tc, Cc).astype(np.float32))


_NC_CACHE = {}


def kernel(x, W_attn, b_attn, W_proj, b_proj):
    from concourse.bass_utils import run_bass_kernel_spmd

    cfg = _cfg_full()
    key = "full"
    if key not in _NC_CACHE:
        _NC_CACHE[key] = build_nc(cfg)
    nc = _NC_CACHE[key]
    in_maps = prep_inputs(cfg, np.asarray(x), np.asarray(W_attn),
                          np.asarray(b_attn), np.asarray(W_proj),
                          np.asarray(b_proj))
    res = run_bass_kernel_spmd(nc, in_maps, list(range(N_CORES)))
    return combine(cfg, res.results, np.asarray(b_proj, dtype=np.float32))


# revision 21
# speedup vs baseline: 1.7338x; 1.0726x over previous
"""Causal self-attention Trainium2 kernel (8-core head-parallel tensor parallel).

Strategy (v3):
  - 16 heads split across 8 cores (2 heads each), all-bf16 PE dataflow.
  - qkv^T = W^T.T @ x^T (bf16), bias added during ACT eviction.
  - Attention per (b, head, q-chunk of 512), feature-major:
      S^T[k,q] = K^T.T @ Q^T  (bf16; diagonal tiles column-trimmed).
      Causal mask: extra PE matmul accumulating Ltri(-1e30) @ I into the
      S PSUM group (no DVE/gpsimd in the dependency chain).
      expS = exp(0.125 * S^T) on ACT -> bf16.
      [y^T; Z] = [V | 1]^T.T @ expS  (PE accumulate; row 64 = Z).
      1/Z = exp(-ln(Z)) on ACT; all activation funcs pinned to the
      natural_log_exp_and_others table set (one ACT_TABLE_LOAD total).
      y^T = psy * bcast(1/Z) (DVE).
  - Normalize + projection emission deferred into later PE streams so the
    PE never idles (keeps the HAM clock warm).
  - out^T partial = Wp^T.T @ y^T -> bf16 partials -> DRAM; host sums.
"""

import sys

if "/opt/trn_rl_repo" not in sys.path:
    sys.path.insert(0, "/opt/trn_rl_repo")

import numpy as np

# ---- problem constants (hardcoded for the grading harness) ----
B, T, C, H = 2, 2048, 1024, 16
HD = C // H            # 64
N_CORES = 8
HPC = H // N_CORES     # heads per core = 2


def _cfg_full():
    return dict(B=B, T=T, C=C, HPC=HPC)


def _make_bacc():
    """Bacc with all activation funcs pinned to the one table set that
    contains both exp and ln, so no ACT_TABLE_LOAD thrash."""
    import concourse.bacc as bacc
    import bass_rust as _bass_rust
    from concourse.hw_specs import get_activation_tables

    class BaccPinnedAct(bacc.Bacc):
        def insert_act_table_loads(self):
            tables = list(get_activation_tables(self.m.arch).items())
            doctored = []
            for name, fns in tables:
                if name == "natural_log_exp_and_others":
                    doctored.append((name, fns))
                else:
                    doctored.append((name, set()))
            _bass_rust.insert_act_table_loads(self, doctored)

    return BaccPinnedAct()


def build_nc(cfg):
    """Build the single-core SPMD Bass program."""
    import concourse.mybir as mybir
    import concourse.tile as tile
    from concourse.masks import make_identity

    Bc, Tc, Cc, hpc = cfg["B"], cfg["T"], cfg["C"], cfg["HPC"]
    f32r = mybir.dt.float32r
    f32 = mybir.dt.float32
    bf16 = mybir.dt.bfloat16
    BT = Bc * Tc
    MQ = hpc * HD                 # rows per m-group (q|k|v) = 128
    assert MQ == 128
    KT_C = Cc // 128              # contraction tiles for qkv/x
    TOKC = 512
    NCH = BT // TOKC              # token chunks over both batches
    QC = Tc // TOKC               # q-chunks per batch
    KTT = Tc // 128               # k-tiles per batch
    MO = Cc // 128                # proj output tiles
    DTILE = TOKC // 128           # 4 diagonal tiles per q-chunk

    nc = _make_bacc()
    xT = nc.declare_dram_parameter("xT", [Cc, BT], bf16, isOutput=False)
    wqkvT = nc.declare_dram_parameter("wqkvT", [Cc, 3 * MQ], bf16, isOutput=False)
    bqkv = nc.declare_dram_parameter("bqkv", [3 * MQ, 1], f32, isOutput=False)
    wpT = nc.declare_dram_parameter("wpT", [MQ, Cc], bf16, isOutput=False)
    outT = nc.declare_dram_parameter("outT", [Cc, BT], bf16, isOutput=True)

    xT_r = xT.rearrange("(kt p) t -> p kt t", p=128)
    wq_r = wqkvT.rearrange("(kt p) m -> p kt m", p=128)
    bq_r = bqkv.rearrange("(g p) o -> p (g o)", p=128)
    outT_r = outT.rearrange("(mo p) t -> p mo t", p=128)

    AF = mybir.ActivationFunctionType

    with tile.TileContext(nc) as tc:
        with (
            tc.tile_pool(name="consts", bufs=1) as consts,
            tc.tile_pool(name="xpool", bufs=3) as xpool,
            tc.tile_pool(name="spool", bufs=5) as spool,
            tc.tile_pool(name="ypool", bufs=3) as ypool,
            tc.tile_pool(name="npool", bufs=3) as npool,
            tc.tile_pool(name="opool", bufs=3) as opool,
            tc.tile_pool(name="ps_mm", bufs=2, space="PSUM") as ps_mm,
            tc.tile_pool(name="ps_y", bufs=2, space="PSUM") as ps_y,
            tc.tile_pool(name="ps_aux", bufs=2, space="PSUM") as ps_aux,
        ):
            # ---- constants; w on sync queue, x on act queue (parallel issue)
            w_sb = consts.tile([128, KT_C, 3 * MQ], bf16, tag="w")
            x_first = xpool.tile([128, KT_C, TOKC], bf16, tag="x")
            for kt in range(KT_C):
                nc.sync.dma_start(out=w_sb[:, kt, :], in_=wq_r[:, kt, :])
                nc.scalar.dma_start(out=x_first[:, kt, :], in_=xT_r[:, kt, 0:TOKC])
            b_sb = consts.tile([128, 3], f32, tag="b")
            nc.sync.dma_start(out=b_sb, in_=bq_r)
            wp_sb = consts.tile([128, Cc], bf16, tag="wp")
            nc.sync.dma_start(out=wp_sb, in_=wpT[:, :])
            ident = consts.tile([128, 128], f32, tag="ident")
            make_identity(nc, ident)
            ident_bf = consts.tile([128, 128], bf16, tag="ident_bf")
            nc.vector.tensor_copy(ident_bf[:, :], ident[:, :])
            # step[p,f] = 1.0 where f >= p else 0 (multiplicative causal
            # mask applied post-exp on DVE over the diagonal 128 cols)
            step = consts.tile([128, 128], bf16, tag="step")
            nc.gpsimd.memset(step[:, :], 1.0)
            nc.gpsimd.affine_select(
                out=step[:, :], in_=step[:, :],
                compare_op=mybir.AluOpType.is_ge,
                fill=0.0,
                base=0,
                pattern=[[1, 128]],
                channel_multiplier=-1,
            )

            ones_row = consts.tile([1, TOKC], f32, tag="ones_row")
            nc.vector.memset(ones_row[:, :], 1.0)
            qT_sb = consts.tile([128, BT], bf16, tag="qT")
            kT_sb = consts.tile([128, BT], bf16, tag="kT")
            vT_sb = consts.tile([128, BT], bf16, tag="vT")

            # V in token-major layout: per b [128(tok), kt, 2*65] where
            # cols hh*65..hh*65+63 = V of head hh, col hh*65+64 = 1.0 (Z row)
            v_sb = [
                consts.tile([128, KTT, 2 * 65], bf16, tag=f"v{b}",
                            name=f"v{b}") for b in range(Bc)
            ]
            for b in range(Bc):
                for hh in range(hpc):
                    nc.vector.memset(v_sb[b][:, :, hh * 65 + 64:hh * 65 + 65], 1.0)

            # ---- phase 1: QKV projection (feature-major, bf16) ----
            for ch in range(NCH):
                if ch == 0:
                    x_t = x_first
                else:
                    x_t = xpool.tile([128, KT_C, TOKC], bf16, tag="x")
                    nc.scalar.dma_start(
                        out=x_t, in_=xT_r[:, :, ch * TOKC:(ch + 1) * TOKC])
                for m in range(3):
                    ps = ps_mm.tile([128, TOKC], f32, tag="mm")
                    for kt in range(KT_C):
                        nc.tensor.matmul(
                            ps[:, :],
                            w_sb[:, kt, m * MQ:(m + 1) * MQ],
                            x_t[:, kt, :],
                            start=(kt == 0), stop=(kt == KT_C - 1),
                        )
                    dst = (qT_sb, kT_sb, vT_sb)[m]
                    nc.scalar.activation(
                        out=dst[:, ch * TOKC:(ch + 1) * TOKC], in_=ps[:, :],
                        func=AF.Identity, bias=b_sb[:, m:m + 1], scale=1.0,
                    )

            # ---- phase 2: V transpose -> token-major v_sb ----
            for b in range(Bc):
                for kt in range(KTT):
                    ps_t = ps_aux.tile([128, 128], bf16, tag="pso",
                                       name="ps_t")
                    nc.tensor.transpose(
                        ps_t[:, :],
                        vT_sb[:, b * Tc + kt * 128:b * Tc + (kt + 1) * 128],
                        ident_bf[:, :],
                    )
                    for hh in range(hpc):
                        nc.vector.tensor_copy(
                            v_sb[b][:, kt, hh * 65:hh * 65 + 64],
                            ps_t[:, hh * HD:(hh + 1) * HD],
                        )

            # ---- phase 3+4: attention + projection, software-pipelined ----
            blocks = [(b, qc) for b in range(Bc) for qc in range(QC)]

            def emit_norm(psy, yT_t, hh):
                """1/Z = exp(-ln Z) on ACT (one pinned table set), broadcast
                across partitions on gpsimd, multiply on DVE."""
                lnZ_t = npool.tile([1, TOKC], f32, tag="lnz")
                nc.scalar.activation(out=lnZ_t[:, :], in_=psy[64:65, :],
                                     func=AF.Ln)
                invz = npool.tile([1, TOKC], f32, tag="invz")
                nc.scalar.activation(out=invz[:, :], in_=lnZ_t[:, :],
                                     func=AF.Exp, scale=-1.0)
                inv_bc = npool.tile([HD, TOKC], f32, tag="invbc")
                nc.gpsimd.partition_broadcast(inv_bc[:, :], invz[0:1, :],
                                              channels=HD)
                with nc.allow_low_precision(reason="yT in bf16"):
                    nc.vector.tensor_mul(
                        yT_t[hh * HD:(hh + 1) * HD, :],
                        psy[0:HD, :], inv_bc[:, :],
                    )

            def make_proj(yT_t, b, qc, last=False):
                """Returns MO small emitters (one proj matmul + evict + DMA
                each) to be scattered across later PE streams."""
                o_t_box = []

                def emit_mo(mo):
                    def emit():
                        if not o_t_box:
                            o_t_box.append(
                                opool.tile([128, MO, TOKC], bf16, tag="o",
                                           name="o_t"))
                        o_t = o_t_box[0]
                        pso = ps_aux.tile([128, TOKC], f32, tag="pso")
                        nc.tensor.matmul(
                            pso[:, :],
                            wp_sb[:, mo * 128:(mo + 1) * 128],
                            yT_t[:, :],
                            start=True, stop=True,
                        )
                        with nc.allow_low_precision(reason="partials in bf16"):
                            if last and mo % 2 == 1:
                                nc.scalar.activation(
                                    out=o_t[:, mo, :], in_=pso[:, :],
                                    func=AF.Copy)
                            else:
                                nc.vector.tensor_copy(o_t[:, mo, :], pso[:, :])
                        nc.sync.dma_start(
                            out=outT_r[:, mo,
                                       b * Tc + qc * TOKC:b * Tc + (qc + 1) * TOKC],
                            in_=o_t[:, mo, :],
                        )
                    return emit
                return [emit_mo(mo) for mo in range(MO)]

            # deferred work queue: emitters injected into later PE streams
            pending = []

            def emit_head(b, qc, hh, yT_t):
                n_kt = (qc + 1) * DTILE
                q0 = b * Tc + qc * TOKC
                psy = ps_y.tile([65, TOKC], f32, tag="y")
                exp_tiles = {}

                def geom(kt):
                    di = kt - qc * DTILE
                    c0 = max(0, di) * 128       # local column start
                    return di, c0, TOKC - c0

                def emit_group(g):
                    # S pair into one double-wide PSUM tile (64-row mode)
                    ps2 = ps_mm.tile([128, 2, TOKC], f32, tag="mm")
                    e2 = spool.tile([128, 2, TOKC], bf16, tag="e")
                    diag = False
                    for sl, kt in enumerate(g):
                        di, c0, W = geom(kt)
                        diag = diag or di >= 0
                        nc.tensor.matmul(
                            ps2[:, sl, 0:W],
                            kT_sb[hh * HD:(hh + 1) * HD,
                                  b * Tc + kt * 128:b * Tc + (kt + 1) * 128],
                            qT_sb[hh * HD:(hh + 1) * HD, q0 + c0:q0 + TOKC],
                            start=True, stop=True,
                        )
                        exp_tiles[kt] = (e2, sl, c0, W)
                    if not diag:
                        # one paired exp over both slots (amortizes the ACT
                        # per-op overhead: ~497ns/tile instead of 570)
                        nc.scalar.activation(out=e2[:, :, :], in_=ps2[:, :, :],
                                             func=AF.Exp, scale=0.125)
                    else:
                        for sl, kt in enumerate(g):
                            di, c0, W = geom(kt)
                            nc.scalar.activation(
                                out=e2[:, sl, 0:W], in_=ps2[:, sl, 0:W],
                                func=AF.Exp, scale=0.125)
                            # causal zeroing post-exp (multiplicative step)
                            nc.vector.tensor_mul(e2[:, sl, 0:128],
                                                 e2[:, sl, 0:128], step[:, :])

                def emit_AV(kt):
                    e2, sl, c0, W = exp_tiles.pop(kt)
                    nc.tensor.matmul(
                        psy[:, c0:TOKC],
                        v_sb[b][:, kt, hh * 65:(hh + 1) * 65],
                        e2[:, sl, 0:W],
                        start=(kt == 0), stop=(kt == n_kt - 1),
                    )

                # groups of 2 k-tiles: [S,S] in 64-row mode, then AVs from
                # two groups back in 128-row mode — few tiling-mode switches
                # and ~2us of exp->AV lookahead so AVs never wait on ACT.
                GS = 2
                groups = [list(range(g, min(g + GS, n_kt)))
                          for g in range(0, n_kt, GS)]
                for gi, g in enumerate(groups):
                    emit_group(g)
                    if gi >= 2:
                        for kt in groups[gi - 2]:
                            emit_AV(kt)
                    # scatter deferred proj matmuls into this PE stream
                    if gi >= 1:
                        for _ in range(2):
                            if pending:
                                pending.pop(0)()
                for g in groups[-2:] if len(groups) >= 2 else groups:
                    for kt in g:
                        emit_AV(kt)
                emit_norm(psy, yT_t, hh)

            prev = None
            for b, qc in blocks:
                yT_t = ypool.tile([128, TOKC], bf16, tag="yT")
                emit_head(b, qc, 0, yT_t)
                if prev is not None:
                    pending.extend(make_proj(*prev))
                emit_head(b, qc, 1, yT_t)
                prev = (yT_t, b, qc)
            pending.extend(make_proj(*prev, last=True))
            while pending:
                pending.pop(0)()

    nc.finalize()
    return nc


def prep_inputs(cfg, x, W_attn, b_attn, W_proj, b_proj):
    """Host-side sharding: returns per-core input dicts."""
    import ml_dtypes
    Bc, Tc, Cc, hpc = cfg["B"], cfg["T"], cfg["C"], cfg["HPC"]
    n_cores = (Cc // HD) // hpc
    BT = Bc * Tc
    MQ = hpc * HD

    x = np.ascontiguousarray(x, dtype=np.float32)
    xT = np.ascontiguousarray(x.reshape(BT, Cc).T).astype(ml_dtypes.bfloat16)

    in_maps = []
    for c in range(n_cores):
        r0 = c * MQ
        rows = []
        for g in range(3):
            rows.append(np.arange(g * Cc + r0, g * Cc + r0 + MQ))
        rows = np.concatenate(rows)
        w_slice = W_attn[rows, :]                       # [384, C]
        wqkvT = np.ascontiguousarray(w_slice.T).astype(ml_dtypes.bfloat16)
        bq = np.ascontiguousarray(b_attn[rows].reshape(MQ * 3, 1))
        wpT = np.ascontiguousarray(W_proj[:, r0:r0 + MQ].T).astype(ml_dtypes.bfloat16)
        in_maps.append({
            "xT": xT,
            "wqkvT": wqkvT,
            "bqkv": bq.astype(np.float32),
            "wpT": wpT,
        })
    return in_maps


def combine(cfg, results, b_proj):
    Bc, Tc, Cc = cfg["B"], cfg["T"], cfg["C"]
    acc = results[0]["outT"].astype(np.float32)
    for r in results[1:]:
        acc += r["outT"].astype(np.float32)
    out = acc.T + b_proj[None, :]
    return np.ascontiguousarray(out.reshape(Bc, Tc, Cc).astype(np.float32))


_NC_CACHE = {}


def kernel(x, W_attn, b_attn, W_proj, b_proj):
    from concourse.bass_utils import run_bass_kernel_spmd

    cfg = _cfg_full()
    key = "full"
    if key not in _NC_CACHE:
        _NC_CACHE[key] = build_nc(cfg)
    nc = _NC_CACHE[key]
    in_maps = prep_inputs(cfg, np.asarray(x), np.asarray(W_attn),
                          np.asarray(b_attn), np.asarray(W_proj),
                          np.asarray(b_proj))
    res = run_bass_kernel_spmd(nc, in_maps, list(range(N_CORES)))
    return combine(cfg, res.results, np.asarray(b_proj, dtype=np.float32))
